# revision 2
# baseline (speedup 1.0000x reference)
"""Trainium2 Bass kernel for nn_Block_57861799412251.

CondBN inverted-residual block:
  1x1 conv (64->192) -> per-sample BN + ReLU
  depthwise 3x3      -> per-sample BN + ReLU
  1x1 conv (192->64) -> per-sample BN
  + identity shortcut -> ReLU

Sharding: data-parallel over batch (32 samples / 8 cores = 4 per core).

Key algebra (per-sample, per-channel BN with gamma>0):
  relu(g*(z-mu)/sd + b) = (g/sd) * relu(z + (sd*b/g - mu))
so each BN+ReLU collapses to a per-channel bias add + relu, with the
positive per-channel scale (g/sd) either cancelled by the next BN's
normalization (BN1, BN2) or folded into the next matmul's weights (BN2
-> proj weights).  BN3's affine is applied in the final residual op.

Layout: channels on partitions, spatial (128*128=16384) on the free axis.
conv1/proj matmuls in float32r, depthwise diag-matmuls in bf16.

The depthwise 3x3 runs as 9 PSUM-accumulated 32x32 diag-matmuls per
32-channel group over a zero-padded [C, 130, 130] bf16 layout, with six
disjoint (row,col) PE tile regions streaming concurrently:
  block0 (ch 0..127):   groups at tiles (0,0) (32,32) (64,64) (96,96)
  block1 (ch 128..191): z1/u1 live on partitions 64..127 (placed there by
  conv1's output-column position); dw tiles (64,0) (96,32) map them back
  to partitions 0..63 where v1/w1/proj live.
"""

import sys

sys.path.insert(0, "/opt/trn_rl_repo")

import numpy as np
import ml_dtypes

BF16 = ml_dtypes.bfloat16

B, CIN, H, W = 32, 64, 128, 128
HW = H * W
CEXP, COUT, D = 192, 64, 6
NCORES = 8
BLOC = B // NCORES  # 4 samples per core
HP, WP = H + 2, W + 2  # padded spatial for depthwise conv
PADHW = HP * WP
EPS = 1e-5
VAR_CORR = HW / (HW - 1.0)  # torch-style unbiased variance
CHUNK = 1024  # PSUM chunk (2 banks)
NCH = HW // CHUNK  # 16
SQP = 2048  # sumsq STT piece
TAPS = [(dy, dx) for dy in (-1, 0, 1) for dx in (-1, 0, 1)]

_PROG = {}


def _build_program(reps=1):
    import concourse.bass as bass
    import concourse.bacc as bacc
    import concourse.tile as tile
    import concourse.mybir as mybir
    from contextlib import ExitStack

    dt = mybir.dt
    AF = mybir.ActivationFunctionType
    OP = mybir.AluOpType

    nc = bacc.Bacc("TRN2", target_bir_lowering=False, debug=False,
                   num_devices=NCORES)

    f32 = dt.float32
    f32r = dt.float32r
    bf = dt.bfloat16

    x_d = nc.dram_tensor("x", [BLOC, CIN, HW], f32, kind="ExternalInput").ap()
    out_d = nc.dram_tensor("out", [BLOC, COUT, HW], f32,
                           kind="ExternalOutput").ap()
    wexp_d = nc.dram_tensor("wexp_lhsT", [CIN, CEXP], bf,
                            kind="ExternalInput").ap()
    # [128, 9, 32]: rows 32a+i hold diag32 of ch-group a (block0)
    dwd0_d = nc.dram_tensor("dw_diag0", [128, 9, 32], bf,
                            kind="ExternalInput").ap()
    # [128, 9, 32]: rows 64+32j+i hold diag32 of ch 128+32j+i (block1)
    dwd1_d = nc.dram_tensor("dw_diag1", [128, 9, 32], bf,
                            kind="ExternalInput").ap()
    wproj0_d = nc.dram_tensor("wproj_lhsT0", [128, COUT], f32,
                              kind="ExternalInput").ap()
    wproj1_d = nc.dram_tensor("wproj_lhsT1", [64, COUT], f32,
                              kind="ExternalInput").ap()
    # per-sample per-channel tables; block0 = ch 0:128, block1 = ch 128:192
    r1b0_d = nc.dram_tensor("r1b0", [BLOC, 128], f32, kind="ExternalInput").ap()
    r1b1_d = nc.dram_tensor("r1b1", [BLOC, 64], f32, kind="ExternalInput").ap()
    r2b0_d = nc.dram_tensor("r2b0", [BLOC, 128], f32, kind="ExternalInput").ap()
    r2b1_d = nc.dram_tensor("r2b1", [BLOC, 64], f32, kind="ExternalInput").ap()
    g2b0_d = nc.dram_tensor("g2b0", [BLOC, 128], f32, kind="ExternalInput").ap()
    g2b1_d = nc.dram_tensor("g2b1", [BLOC, 64], f32, kind="ExternalInput").ap()
    g3_d = nc.dram_tensor("g3", [BLOC, COUT], f32, kind="ExternalInput").ap()
    b3_d = nc.dram_tensor("b3", [BLOC, COUT], f32, kind="ExternalInput").ap()

    with ExitStack() as ctx:
        tc = ctx.enter_context(tile.TileContext(nc))
        const = ctx.enter_context(tc.tile_pool(name="const", bufs=1))
        stats = ctx.enter_context(tc.tile_pool(name="stats", bufs=2))
        big = ctx.enter_context(tc.tile_pool(name="big", bufs=1))
        xin = ctx.enter_context(tc.tile_pool(name="xin", bufs=4))
        psum = ctx.enter_context(tc.tile_pool(name="psum", bufs=4,
                                              space="PSUM"))

        # ---- constants ----
        wexp_sb = const.tile([CIN, CEXP], bf)
        nc.sync.dma_start(out=wexp_sb, in_=wexp_d)
        dwd0_sb = const.tile([128, 9, 32], bf)
        nc.sync.dma_start(out=dwd0_sb, in_=dwd0_d)
        dwd1_sb = const.tile([128, 9, 32], bf)
        nc.sync.dma_start(out=dwd1_sb, in_=dwd1_d)
        wproj0_sb = const.tile([128, COUT], f32)
        nc.sync.dma_start(out=wproj0_sb, in_=wproj0_d)
        wproj1_sb = const.tile([64, COUT], f32)
        nc.sync.dma_start(out=wproj1_sb, in_=wproj1_d)
        eps_sb = const.tile([128, 1], f32)
        nc.vector.memset(eps_sb, EPS)

        # padded u buffers (borders stay zero forever).
        # u0: ch 0..127 on partitions 0..127; u1: ch 128..191 on 64..127.
        u0_sb = const.tile([128, PADHW], bf)
        nc.gpsimd.memset(u0_sb, 0.0)
        u1_sb = const.tile([128, PADHW], bf)
        nc.gpsimd.memset(u1_sb, 0.0)
        u0v = u0_sb.rearrange("p (h w) -> p h w", h=HP)
        u1v = u1_sb.rearrange("p (h w) -> p h w", h=HP)

        loop_ctx = tc.For_i(0, reps, 1) if reps > 1 else None
        if loop_ctx is not None:
            ctx.enter_context(loop_ctx)

        def chunk_sumsq(name, src, engine, piece=SQP):
            """accumulate sum(src^2) over the free axis via chunked STT.
            src: [P, HW] bf16 AP (any base partition); all operand tiles
            are placed at src's base partition."""
            P = src.partition_size()
            lo = src.base_partition()
            n = HW // piece
            acc = stats.tile([128, n], f32, tag=f"{name}_acc",
                             name=f"{name}_acc")[lo:lo + P]
            for i in range(n):
                scr = big.tile([128, piece], bf, tag="scr", bufs=1,
                               name=f"{name}_scr")
                engine.scalar_tensor_tensor(
                    out=scr[lo:lo + P],
                    in0=src[:, i * piece:(i + 1) * piece],
                    scalar=1.0,
                    in1=src[:, i * piece:(i + 1) * piece],
                    op0=OP.bypass,
                    op1=OP.mult,
                    accum_out=acc[:, i:i + 1],
                )
            tot = stats.tile([128, 1], f32, tag=f"{name}_tot",
                             name=f"{name}_tot")[lo:lo + P]
            nc.vector.tensor_reduce(tot, acc, axis=mybir.AxisListType.X,
                                    op=OP.add)
            return tot

        def bn_prep(name, sum_parts, sumsq, eps_ap, r_ap):
            """Produce (c = sd*r - mean, rstd, mean) for a [P,1] stat lane
            set. sum_parts: [P, n] per-chunk sums; sumsq: [P,1].  All tiles
            are placed at sum_parts' base partition."""
            P = sum_parts.partition_size()
            lo = sum_parts.base_partition()

            def stile(suffix):
                return stats.tile([128, 1], f32, tag=f"{name}_{suffix}",
                                  name=f"{name}_{suffix}")[lo:lo + P]

            s = stile("s")
            nc.vector.tensor_reduce(s, sum_parts, axis=mybir.AxisListType.X,
                                    op=OP.add)
            mean = stile("mean")
            nc.vector.tensor_scalar(out=mean, in0=s, scalar1=1.0 / HW,
                                    scalar2=None, op0=OP.mult)
            ex2 = stile("ex2")
            nc.vector.tensor_scalar(out=ex2, in0=sumsq, scalar1=1.0 / HW,
                                    scalar2=None, op0=OP.mult)
            var = stile("var")
            nc.vector.scalar_tensor_tensor(out=var, in0=mean, scalar=mean,
                                           in1=ex2, op0=OP.mult,
                                           op1=OP.subtract)
            nc.vector.tensor_scalar(out=var, in0=var, scalar1=-1.0,
                                    scalar2=None, op0=OP.mult)
            sd = stile("sd")
            nc.scalar.activation(out=sd, in_=var, func=AF.Sqrt,
                                 bias=eps_ap, scale=VAR_CORR)
            rstd = stile("rstd")
            nc.vector.reciprocal(rstd, sd)
            c = stile("c")
            nc.vector.scalar_tensor_tensor(out=c, in0=sd, scalar=r_ap,
                                           in1=mean, op0=OP.mult,
                                           op1=OP.subtract)
            return c, rstd, mean

        for s in range(BLOC):
            # ---- per-sample params.  Block1 z-side tables live on
            # partitions 64..127; v-side (r2/g2) on 0..63. ----
            def ld(name, dram_ap, lo, P):
                t = stats.tile([128, 1], f32, tag=f"p_{name}",
                               name=f"p_{name}")
                nc.sync.dma_start(out=t[lo:lo + P], in_=dram_ap[s, :, None])
                return t[lo:lo + P]

            r1b0 = ld("r1b0", r1b0_d, 0, 128)
            r1b1 = ld("r1b1", r1b1_d, 64, 64)   # z-side: partitions 64..127
            r2b0 = ld("r2b0", r2b0_d, 0, 128)
            r2b1 = ld("r2b1", r2b1_d, 0, 64)    # v-side: partitions 0..63
            g2b0 = ld("g2b0", g2b0_d, 0, 128)
            g2b1 = ld("g2b1", g2b1_d, 0, 64)
            g3 = ld("g3", g3_d, 0, COUT)
            b3t = ld("b3", b3_d, 0, COUT)

            # ---- phase A: conv1 (f32r, straight from streamed x) ----
            z0 = big.tile([128, HW], bf, tag="zv0", name="z0")
            z1t = big.tile([128, HW], bf, tag="zv1", name="z1t")
            z1 = z1t[64:128]  # ch 128..191 on partitions 64..127
            sumz0 = stats.tile([128, NCH], f32, tag="sumz0", name="sumz0")
            sumz1 = stats.tile([128, NCH], f32, tag="sumz1", name="sumz1")
            for c in range(NCH):  # 16 chunks of 1024
                xp = xin.tile([CIN, CHUNK], f32, tag="xp", name="xp")
                nc.sync.dma_start(out=xp,
                                  in_=x_d[s, :, c * CHUNK:(c + 1) * CHUNK])
                xbf = xin.tile([CIN, CHUNK], bf, tag="xbf", name="xbf")
                nc.gpsimd.tensor_copy(xbf, xp)
                pz0 = psum.tile([128, CHUNK], f32, tag="ps", name="pz0")
                pz1 = psum.tile([128, CHUNK], f32, tag="ps", name="pz1")
                for k in range(CHUNK // 512):
                    rhs = xbf[:, k * 512:(k + 1) * 512]
                    nc.tensor.matmul(pz0[:, k * 512:(k + 1) * 512],
                                     wexp_sb[:, 0:128], rhs,
                                     start=True, stop=True,
                                     tile_position=(0, 0))
                    nc.tensor.matmul(pz1[64:128, k * 512:(k + 1) * 512],
                                     wexp_sb[:, 128:CEXP], rhs,
                                     start=True, stop=True,
                                     tile_position=(0, 64))
                sl = slice(c * CHUNK, (c + 1) * CHUNK)
                nc.scalar.activation(out=z0[:, sl], in_=pz0, func=AF.Copy,
                                     accum_out=sumz0[:, c:c + 1])
                nc.scalar.activation(out=z1[:, sl], in_=pz1[64:128],
                                     func=AF.Copy,
                                     accum_out=sumz1[64:128, c:c + 1])

            # ---- phase B: BN1 stats -> c1; u = relu(z + c1) ----
            sq_z0 = chunk_sumsq("sqz0", z0, nc.vector)
            sq_z1 = chunk_sumsq("sqz1", z1, nc.vector)
            c1_0, _, _ = bn_prep("bn1b0", sumz0, sq_z0, eps_sb[0:128], r1b0)
            c1_1, _, _ = bn_prep("bn1b1", sumz1[64:128], sq_z1,
                                 eps_sb[64:128], r1b1)

            z0v = z0.rearrange("p (h w) -> p h w", h=H)
            z1v = z1.rearrange("p (h w) -> p h w", h=H)
            nc.gpsimd.tensor_scalar(out=u0v[:, 1:H + 1, 1:W + 1], in0=z0v,
                                    scalar1=c1_0, scalar2=0.0,
                                    op0=OP.add, op1=OP.max)
            nc.gpsimd.tensor_scalar(out=u1v[64:128, 1:H + 1, 1:W + 1],
                                    in0=z1v, scalar1=c1_1, scalar2=0.0,
                                    op0=OP.add, op1=OP.max)

            # ---- phase C: depthwise 3x3 -> v, 6 concurrent PE tile slots ----
            v0 = big.tile([128, HW], bf, tag="zv0", name="v0")
            v1t = big.tile([128, HW], bf, tag="zv1", name="v1t")
            v1 = v1t[0:64]  # ch 128..191 back on partitions 0..63
            sumv0 = stats.tile([128, NCH], f32, tag="sumv0", name="sumv0")
            sumv1 = stats.tile([128, NCH], f32, tag="sumv1", name="sumv1")
            rows_per_512 = 512 // W  # 4
            for c in range(NCH):
                pv0 = psum.tile([128, CHUNK], f32, tag="ps", name="pv0")
                pv1 = psum.tile([64, CHUNK], f32, tag="ps", name="pv1")
                for k in range(CHUNK // 512):
                    h0 = (c * CHUNK + k * 512) // W
                    ksl = slice(k * 512, (k + 1) * 512)
                    for ti, (dy, dx) in enumerate(TAPS):
                        rsl = slice(1 + h0 + dy, 1 + h0 + dy + rows_per_512)
                        csl = slice(1 + dx, 1 + dx + W)
                        st, sp = (ti == 0), (ti == 8)
                        for a in range(4):  # block0 groups: tiles (32a, 32a)
                            pa = slice(32 * a, 32 * a + 32)
                            nc.tensor.matmul(
                                pv0[pa, ksl], dwd0_sb[pa, ti, :],
                                u0v[pa, rsl, csl],
                                start=st, stop=sp,
                                tile_position=(32 * a, 32 * a))
                        for j in range(2):  # block1: tiles (64+32j, 32j)
                            pr = slice(64 + 32 * j, 96 + 32 * j)
                            po_ = slice(32 * j, 32 * j + 32)
                            nc.tensor.matmul(
                                pv1[po_, ksl], dwd1_sb[pr, ti, :],
                                u1v[pr, rsl, csl],
                                start=st, stop=sp,
                                tile_position=(64 + 32 * j, 32 * j))
                sl = slice(c * CHUNK, (c + 1) * CHUNK)
                nc.scalar.activation(out=v0[:, sl], in_=pv0, func=AF.Copy,
                                     accum_out=sumv0[:, c:c + 1])
                nc.scalar.activation(out=v1[:, sl], in_=pv1, func=AF.Copy,
                                     accum_out=sumv1[0:64, c:c + 1])

            # ---- phase D: BN2 -> c2; w = relu(v + c2) in-place;
            #      proj weights scaled by g2*rstd_v ----
            sq_v0 = chunk_sumsq("sqv0", v0, nc.vector)
            sq_v1 = chunk_sumsq("sqv1", v1, nc.vector)
            c2_0, rstdv0, _ = bn_prep("bn2b0", sumv0, sq_v0, eps_sb[0:128],
                                      r2b0)
            c2_1, rstdv1, _ = bn_prep("bn2b1", sumv1[0:64], sq_v1,
                                      eps_sb[0:64], r2b1)

            alpha0 = stats.tile([128, 1], f32, tag="alpha0", name="alpha0")
            nc.vector.tensor_mul(alpha0, g2b0, rstdv0)
            alpha1 = stats.tile([64, 1], f32, tag="alpha1", name="alpha1")
            nc.vector.tensor_mul(alpha1, g2b1, rstdv1)
            projs0 = stats.tile([128, COUT], bf, tag="projs0", name="projs0")
            nc.scalar.activation(out=projs0, in_=wproj0_sb, func=AF.Copy,
                                 scale=alpha0)
            projs1 = stats.tile([64, COUT], bf, tag="projs1", name="projs1")
            nc.scalar.activation(out=projs1, in_=wproj1_sb, func=AF.Copy,
                                 scale=alpha1)

            nc.gpsimd.tensor_scalar(out=v0, in0=v0, scalar1=c2_0,
                                    scalar2=0.0, op0=OP.add, op1=OP.max)
            nc.gpsimd.tensor_scalar(out=v1, in0=v1, scalar1=c2_1,
                                    scalar2=0.0, op0=OP.add, op1=OP.max)

            # ---- phase E: proj conv (bf16) -> out3, evict + sums ----
            out3 = big.tile([64, HW], bf, tag="out3", name="out3")
            sumo = stats.tile([64, NCH], f32, tag="sumo", name="sumo")
            for c in range(NCH):
                po = psum.tile([64, CHUNK], f32, tag="ps", name="po")
                for k in range(CHUNK // 512):
                    sl = slice(c * CHUNK + k * 512, c * CHUNK + (k + 1) * 512)
                    nc.tensor.matmul(po[:, k * 512:(k + 1) * 512], projs0,
                                     v0[:, sl], start=True, stop=False,
                                     tile_position=(0, 0))
                    nc.tensor.matmul(po[:, k * 512:(k + 1) * 512], projs1,
                                     v1[:, sl], start=False, stop=True,
                                     tile_position=(0, 0))
                nc.scalar.activation(out=out3[:, c * CHUNK:(c + 1) * CHUNK],
                                     in_=po, func=AF.Copy,
                                     accum_out=sumo[:, c:c + 1])

            # ---- phase F: BN3 stats; final = relu(a3*out3 + b3 + x) ----
            sq_o = chunk_sumsq("sqo", out3, nc.vector)
            _, rstd3, mean3 = bn_prep("bn3", sumo, sq_o, eps_sb[0:64], g3)
            a3 = stats.tile([COUT, 1], f32, tag="a3", name="a3")
            nc.vector.tensor_mul(a3, g3, rstd3)
            t3 = stats.tile([COUT, 1], f32, tag="t3", name="t3")
            nc.vector.tensor_mul(t3, mean3, a3)
            b3f = stats.tile([COUT, 1], f32, tag="b3f", name="b3f")
            nc.vector.tensor_tensor(b3f, b3t, t3, op=OP.subtract)

            for c in range(NCH):
                xr = xin.tile([COUT, CHUNK], f32, tag="xp", name="xr")
                sl = slice(c * CHUNK, (c + 1) * CHUNK)
                nc.sync.dma_start(out=xr, in_=x_d[s, :, sl])
                nc.vector.affine_then_add(out=xr, in0=out3[:, sl], in1=xr,
                                          scale=a3, bias=b3f)
                nc.scalar.activation(out=xr, in_=xr, func=AF.Relu)
                nc.sync.dma_start(out=out_d[s, :, sl], in_=xr)

    nc.compile()
    return nc


def _get_program(reps=1):
    key = ("nc", reps)
    if key not in _PROG:
        _PROG[key] = _build_program(reps)
    return _PROG[key]


def _host_prep(x, device_ids, w_exp, g_exp, b_exp, w_dw, g_dw, b_dw,
               w_proj, g_proj, b_proj):
    """Build the per-core input maps (numpy only)."""
    f32 = np.float32
    ids = np.asarray(device_ids)
    ge = np.asarray(g_exp, f32)[:, :, 0, 0]   # [D, 192]
    be = np.asarray(b_exp, f32)[:, :, 0, 0]
    gd = np.asarray(g_dw, f32)[:, :, 0, 0]
    bd = np.asarray(b_dw, f32)[:, :, 0, 0]
    gp = np.asarray(g_proj, f32)[:, :, 0, 0]  # [D, 64]
    bp = np.asarray(b_proj, f32)[:, :, 0, 0]
    assert (ge > 0).all() and (gd > 0).all(), "relu-commute needs gamma>0"

    r1 = (be / ge)[ids]   # [B, 192]
    r2 = (bd / gd)[ids]
    g2 = gd[ids]
    g3 = gp[ids]          # [B, 64]
    b3 = bp[ids]

    wexp_lhsT = np.ascontiguousarray(
        np.asarray(w_exp, f32)[:, :, 0, 0].T).astype(BF16)  # [64, 192]
    wp = np.asarray(w_proj, f32)[:, :, 0, 0]  # [64, 192]
    wproj_lhsT0 = np.ascontiguousarray(wp[:, 0:128].T).astype(f32)  # [128,64]
    wproj_lhsT1 = np.ascontiguousarray(wp[:, 128:192].T).astype(f32)  # [64,64]

    dw = np.asarray(w_dw, f32)[:, 0, :, :]  # [192, 3, 3]
    dw_diag0 = np.zeros((128, 9, 32), f32)
    dw_diag1 = np.zeros((128, 9, 32), f32)
    ii = np.arange(32)
    for ti, (dy, dx) in enumerate(TAPS):
        for a in range(4):
            dw_diag0[32 * a + ii, ti, ii] = dw[32 * a + ii, dy + 1, dx + 1]
        for j in range(2):
            dw_diag1[64 + 32 * j + ii, ti, ii] = dw[128 + 32 * j + ii,
                                                    dy + 1, dx + 1]
    dw_diag0 = dw_diag0.astype(BF16)
    dw_diag1 = dw_diag1.astype(BF16)

    xf = np.asarray(x, f32).reshape(B, CIN, HW)

    in_maps = []
    for core in range(NCORES):
        sl = slice(core * BLOC, (core + 1) * BLOC)
        in_maps.append({
            "x": np.ascontiguousarray(xf[sl]),
            "wexp_lhsT": wexp_lhsT,
            "dw_diag0": dw_diag0,
            "dw_diag1": dw_diag1,
            "wproj_lhsT0": wproj_lhsT0,
            "wproj_lhsT1": wproj_lhsT1,
            "r1b0": np.ascontiguousarray(r1[sl, 0:128], f32),
            "r1b1": np.ascontiguousarray(r1[sl, 128:192], f32),
            "r2b0": np.ascontiguousarray(r2[sl, 0:128], f32),
            "r2b1": np.ascontiguousarray(r2[sl, 128:192], f32),
            "g2b0": np.ascontiguousarray(g2[sl, 0:128], f32),
            "g2b1": np.ascontiguousarray(g2[sl, 128:192], f32),
            "g3": np.ascontiguousarray(g3[sl], f32),
            "b3": np.ascontiguousarray(b3[sl], f32),
        })
    return in_maps


def kernel(x, device_ids, w_exp, g_exp, b_exp, w_dw, g_dw, b_dw,
           w_proj, g_proj, b_proj, _trace=False, _tmpdir=None):
    from concourse import bass_utils

    nc = _get_program()
    in_maps = _host_prep(x, device_ids, w_exp, g_exp, b_exp, w_dw, g_dw,
                         b_dw, w_proj, g_proj, b_proj)
    res = bass_utils.run_bass_kernel_spmd(
        nc, in_maps, core_ids=list(range(NCORES)), trace=_trace,
        tmpdir=_tmpdir)
    out = np.stack([r["out"] for r in res.results], axis=0)
    out = out.reshape(B, COUT, H, W).astype(np.float32)
    if _trace:
        kernel._last_results = res
    return out



# revision 10
# speedup vs baseline: 2.3338x; 2.3338x over previous
"""Trainium2 Bass kernel for nn_Block_57861799412251.

CondBN inverted-residual block:
  1x1 conv (64->192) -> per-sample BN + ReLU
  depthwise 3x3      -> per-sample BN + ReLU
  1x1 conv (192->64) -> per-sample BN
  + identity shortcut -> ReLU

Sharding: data-parallel over batch (32 samples / 8 cores = 4 per core).

Key algebra (per-sample, per-channel BN with gamma>0):
  relu(g*(z-mu)/sd + b) = (g/sd) * relu(z + (sd*b/g - mu))
so each BN+ReLU collapses to a per-channel bias add + relu, with the
positive per-channel scale (g/sd) either cancelled by the next BN's
normalization (BN1, BN2) or folded into the next matmul's weights (BN2
-> proj weights).  BN3's affine is applied in the final residual op.

Layout: channels on partitions, spatial (128*128=16384) on the free axis.
conv1/proj matmuls in float32r, depthwise diag-matmuls in bf16.

The depthwise 3x3 runs as 9 PSUM-accumulated 32x32 diag-matmuls per
32-channel group over a zero-padded [C, 130, 130] bf16 layout, with six
disjoint (row,col) PE tile regions streaming concurrently:
  block0 (ch 0..127):   groups at tiles (0,0) (32,32) (64,64) (96,96)
  block1 (ch 128..191): z1/u1 live on partitions 64..127 (placed there by
  conv1's output-column position); dw tiles (64,0) (96,32) map them back
  to partitions 0..63 where v1/w1/proj live.
"""

import sys

sys.path.insert(0, "/opt/trn_rl_repo")

import numpy as np
import ml_dtypes

BF16 = ml_dtypes.bfloat16

B, CIN, H, W = 32, 64, 128, 128
HW = H * W
CEXP, COUT, D = 192, 64, 6
NCORES = 8
BLOC = B // NCORES  # 4 samples per core
HP, WP = H + 2, W + 2  # padded spatial for depthwise conv
PADHW = HP * WP
EPS = 1e-5
VAR_CORR = HW / (HW - 1.0)  # torch-style unbiased variance
CHUNK = 1024  # PSUM chunk (2 banks)
NCH = HW // CHUNK  # 16
SQP = 2048  # sumsq STT piece
TAPS = [(dy, dx) for dy in (-1, 0, 1) for dx in (-1, 0, 1)]

_PROG = {}


def _build_program(reps=1):
    import concourse.bass as bass
    import concourse.bacc as bacc
    import concourse.tile as tile
    import concourse.mybir as mybir
    from contextlib import ExitStack

    dt = mybir.dt
    AF = mybir.ActivationFunctionType
    OP = mybir.AluOpType

    nc = bacc.Bacc("TRN2", target_bir_lowering=False, debug=False,
                   num_devices=NCORES)

    f32 = dt.float32
    f32r = dt.float32r
    bf = dt.bfloat16

    x_d = nc.dram_tensor("x", [BLOC, CIN, HW], f32, kind="ExternalInput").ap()
    out_d = nc.dram_tensor("out", [BLOC, COUT, HW], f32,
                           kind="ExternalOutput").ap()
    wexp_d = nc.dram_tensor("wexp_lhsT", [CIN, CEXP], bf,
                            kind="ExternalInput").ap()
    # full-width diag matrices per tap: [c, t, c] = w_dw[c, tap t]
    dwd0_d = nc.dram_tensor("dw_diagF0", [128, 9, 128], bf,
                            kind="ExternalInput").ap()
    # block1 diag64 per tap: [c, t, c] = w_dw[128+c, tap t]
    dwd1_d = nc.dram_tensor("dw_diagF1", [64, 9, 64], bf,
                            kind="ExternalInput").ap()
    wproj0_d = nc.dram_tensor("wproj_lhsT0", [128, COUT], f32,
                              kind="ExternalInput").ap()
    wproj1_d = nc.dram_tensor("wproj_lhsT1", [64, COUT], f32,
                              kind="ExternalInput").ap()
    # per-sample per-channel tables; block0 = ch 0:128, block1 = ch 128:192
    r1b0_d = nc.dram_tensor("r1b0", [BLOC, 128], f32, kind="ExternalInput").ap()
    r1b1_d = nc.dram_tensor("r1b1", [BLOC, 64], f32, kind="ExternalInput").ap()
    r2b0_d = nc.dram_tensor("r2b0", [BLOC, 128], f32, kind="ExternalInput").ap()
    r2b1_d = nc.dram_tensor("r2b1", [BLOC, 64], f32, kind="ExternalInput").ap()
    g2b0_d = nc.dram_tensor("g2b0", [BLOC, 128], f32, kind="ExternalInput").ap()
    g2b1_d = nc.dram_tensor("g2b1", [BLOC, 64], f32, kind="ExternalInput").ap()
    g3_d = nc.dram_tensor("g3", [BLOC, COUT], f32, kind="ExternalInput").ap()
    b3_d = nc.dram_tensor("b3", [BLOC, COUT], f32, kind="ExternalInput").ap()

    with ExitStack() as ctx:
        tc = ctx.enter_context(tile.TileContext(nc))
        const = ctx.enter_context(tc.tile_pool(name="const", bufs=1))
        stats = ctx.enter_context(tc.tile_pool(name="stats", bufs=2))
        big = ctx.enter_context(tc.tile_pool(name="big", bufs=1))
        xin = ctx.enter_context(tc.tile_pool(name="xin", bufs=4))
        psum = ctx.enter_context(tc.tile_pool(name="psum", bufs=4,
                                              space="PSUM"))

        # ---- constants ----
        wexp_sb = const.tile([CIN, CEXP], bf)
        nc.sync.dma_start(out=wexp_sb, in_=wexp_d)
        dwd0_sb = const.tile([128, 9, 128], bf)
        nc.sync.dma_start(out=dwd0_sb, in_=dwd0_d)
        # block1 diags live at partitions 64..127 (tile row base 64)
        dwd1_sb = const.tile([128, 9, 64], bf)
        nc.sync.dma_start(out=dwd1_sb[64:128], in_=dwd1_d)
        wproj0_sb = const.tile([128, COUT], f32)
        nc.sync.dma_start(out=wproj0_sb, in_=wproj0_d)
        wproj1_sb = const.tile([64, COUT], f32)
        nc.sync.dma_start(out=wproj1_sb, in_=wproj1_d)
        eps_sb = const.tile([128, 1], f32)
        nc.vector.memset(eps_sb, EPS)

        # padded u buffers (borders stay zero forever).
        # u0: ch 0..127 on partitions 0..127; u1: ch 128..191 on 64..127.
        u0_sb = const.tile([128, PADHW], bf)
        nc.gpsimd.memset(u0_sb, 0.0)
        u1_sb = const.tile([128, PADHW], bf)
        nc.gpsimd.memset(u1_sb, 0.0)
        u0v = u0_sb.rearrange("p (h w) -> p h w", h=HP)
        u1v = u1_sb.rearrange("p (h w) -> p h w", h=HP)

        loop_ctx = tc.For_i(0, reps, 1) if reps > 1 else None
        if loop_ctx is not None:
            ctx.enter_context(loop_ctx)

        def chunk_sumsq(name, src, engine, piece=SQP):
            """accumulate sum(src^2) over the free axis via chunked STT.
            src: [P, HW] bf16 AP (any base partition); all operand tiles
            are placed at src's base partition."""
            P = src.partition_size()
            lo = src.base_partition()
            n = HW // piece
            acc = stats.tile([128, n], f32, tag=f"{name}_acc",
                             name=f"{name}_acc")[lo:lo + P]
            for i in range(n):
                scr = big.tile([128, piece], bf, tag="scr", bufs=1,
                               name=f"{name}_scr")
                engine.scalar_tensor_tensor(
                    out=scr[lo:lo + P],
                    in0=src[:, i * piece:(i + 1) * piece],
                    scalar=1.0,
                    in1=src[:, i * piece:(i + 1) * piece],
                    op0=OP.bypass,
                    op1=OP.mult,
                    accum_out=acc[:, i:i + 1],
                )
            tot = stats.tile([128, 1], f32, tag=f"{name}_tot",
                             name=f"{name}_tot")[lo:lo + P]
            nc.vector.tensor_reduce(tot, acc, axis=mybir.AxisListType.X,
                                    op=OP.add)
            return tot

        def bn_prep(name, sum_parts, sumsq, eps_ap, r_ap):
            """Produce (c = sd*r - mean, rstd, mean) for a [P,1] stat lane
            set. sum_parts: [P, n] per-chunk sums; sumsq: [P,1].  All tiles
            are placed at sum_parts' base partition."""
            P = sum_parts.partition_size()
            lo = sum_parts.base_partition()

            def stile(suffix):
                return stats.tile([128, 1], f32, tag=f"{name}_{suffix}",
                                  name=f"{name}_{suffix}")[lo:lo + P]

            s = stile("s")
            nc.vector.tensor_reduce(s, sum_parts, axis=mybir.AxisListType.X,
                                    op=OP.add)
            mean = stile("mean")
            nc.vector.tensor_scalar(out=mean, in0=s, scalar1=1.0 / HW,
                                    scalar2=None, op0=OP.mult)
            ex2 = stile("ex2")
            nc.vector.tensor_scalar(out=ex2, in0=sumsq, scalar1=1.0 / HW,
                                    scalar2=None, op0=OP.mult)
            var = stile("var")
            nc.vector.scalar_tensor_tensor(out=var, in0=mean, scalar=mean,
                                           in1=ex2, op0=OP.mult,
                                           op1=OP.subtract)
            nc.vector.tensor_scalar(out=var, in0=var, scalar1=-1.0,
                                    scalar2=None, op0=OP.mult)
            sd = stile("sd")
            nc.scalar.activation(out=sd, in_=var, func=AF.Sqrt,
                                 bias=eps_ap, scale=VAR_CORR)
            rstd = stile("rstd")
            nc.vector.reciprocal(rstd, sd)
            c = stile("c")
            nc.vector.scalar_tensor_tensor(out=c, in0=sd, scalar=r_ap,
                                           in1=mean, op0=OP.mult,
                                           op1=OP.subtract)
            return c, rstd, mean

        for s in range(BLOC):
            # ---- per-sample params.  Block1 z-side tables live on
            # partitions 64..127; v-side (r2/g2) on 0..63. ----
            def ld(name, dram_ap, lo, P):
                t = stats.tile([128, 1], f32, tag=f"p_{name}",
                               name=f"p_{name}")
                nc.sync.dma_start(out=t[lo:lo + P], in_=dram_ap[s, :, None])
                return t[lo:lo + P]

            r1b0 = ld("r1b0", r1b0_d, 0, 128)
            r1b1 = ld("r1b1", r1b1_d, 64, 64)   # z-side: partitions 64..127
            r2b0 = ld("r2b0", r2b0_d, 0, 128)
            r2b1 = ld("r2b1", r2b1_d, 0, 64)    # v-side: partitions 0..63
            g2b0 = ld("g2b0", g2b0_d, 0, 128)
            g2b1 = ld("g2b1", g2b1_d, 0, 64)
            g3 = ld("g3", g3_d, 0, COUT)
            b3t = ld("b3", b3_d, 0, COUT)

            # ---- phase A: conv1 (f32r, straight from streamed x) ----
            z0 = big.tile([128, HW], bf, tag="zv0", name="z0")
            z1t = big.tile([128, HW], bf, tag="zv1", name="z1t")
            z1 = z1t[64:128]  # ch 128..191 on partitions 64..127
            sumz0 = stats.tile([128, NCH], f32, tag="sumz0", name="sumz0")
            sumz1 = stats.tile([128, NCH], f32, tag="sumz1", name="sumz1")
            for c in range(NCH):  # 16 chunks of 1024
                xp = xin.tile([CIN, CHUNK], f32, tag="xp", name="xp")
                nc.sync.dma_start(out=xp,
                                  in_=x_d[s, :, c * CHUNK:(c + 1) * CHUNK])
                xbf = xin.tile([CIN, CHUNK], bf, tag="xbf", name="xbf")
                nc.scalar.activation(out=xbf, in_=xp, func=AF.Copy)
                pz0 = psum.tile([128, CHUNK], f32, tag="ps", name="pz0")
                pz1 = psum.tile([128, CHUNK], f32, tag="ps", name="pz1")
                for k in range(CHUNK // 512):
                    rhs = xbf[:, k * 512:(k + 1) * 512]
                    nc.tensor.matmul(pz0[:, k * 512:(k + 1) * 512],
                                     wexp_sb[:, 0:128], rhs,
                                     start=True, stop=True,
                                     tile_position=(0, 0))
                    nc.tensor.matmul(pz1[64:128, k * 512:(k + 1) * 512],
                                     wexp_sb[:, 128:CEXP], rhs,
                                     start=True, stop=True,
                                     tile_position=(0, 64))
                sl = slice(c * CHUNK, (c + 1) * CHUNK)
                nc.scalar.activation(out=z0[:, sl], in_=pz0, func=AF.Copy,
                                     accum_out=sumz0[:, c:c + 1])
                nc.scalar.activation(out=z1[:, sl], in_=pz1[64:128],
                                     func=AF.Copy,
                                     accum_out=sumz1[64:128, c:c + 1])

            # ---- phase B: BN1 stats -> c1; u = relu(z + c1) ----
            sq_z0 = chunk_sumsq("sqz0", z0, nc.vector)
            sq_z1 = chunk_sumsq("sqz1", z1, nc.vector)
            c1_0, _, _ = bn_prep("bn1b0", sumz0, sq_z0, eps_sb[0:128], r1b0)
            c1_1, _, _ = bn_prep("bn1b1", sumz1[64:128], sq_z1,
                                 eps_sb[64:128], r1b1)

            z0v = z0.rearrange("p (h w) -> p h w", h=H)
            z1v = z1.rearrange("p (h w) -> p h w", h=H)
            for q in range(4):  # 32-row slabs on DVE (4x perf mode)
                rs = slice(1 + 32 * q, 1 + 32 * (q + 1))
                zs = slice(32 * q, 32 * (q + 1))
                nc.vector.tensor_scalar(out=u0v[:, rs, 1:W + 1],
                                        in0=z0v[:, zs, :],
                                        scalar1=c1_0, scalar2=0.0,
                                        op0=OP.add, op1=OP.max)
                nc.vector.tensor_scalar(out=u1v[64:128, rs, 1:W + 1],
                                        in0=z1v[:, zs, :],
                                        scalar1=c1_1, scalar2=0.0,
                                        op0=OP.add, op1=OP.max)

            # ---- phase C: depthwise 3x3 -> v, 6 concurrent PE tile slots ----
            v0 = big.tile([128, HW], bf, tag="zv0", name="v0")
            v1t = big.tile([128, HW], bf, tag="zv1", name="v1t")
            v1 = v1t[0:64]  # ch 128..191 back on partitions 0..63
            sumv0 = stats.tile([128, NCH], f32, tag="sumv0", name="sumv0")
            sumv1 = stats.tile([128, NCH], f32, tag="sumv1", name="sumv1")
            rows_per_512 = 512 // W  # 4
            for c in range(NCH):
                pv0 = psum.tile([128, CHUNK], f32, tag="ps", name="pv0")
                pv1 = psum.tile([64, CHUNK], f32, tag="ps", name="pv1")
                for k in range(CHUNK // 512):
                    h0 = (c * CHUNK + k * 512) // W
                    ksl = slice(k * 512, (k + 1) * 512)
                    for ti, (dy, dx) in enumerate(TAPS):
                        rsl = slice(1 + h0 + dy, 1 + h0 + dy + rows_per_512)
                        csl = slice(1 + dx, 1 + dx + W)
                        st, sp = (ti == 0), (ti == 8)
                        # block0: one full 128-wide diag matmul at (0,0)
                        nc.tensor.matmul(
                            pv0[:, ksl], dwd0_sb[:, ti, :],
                            u0v[:, rsl, csl],
                            start=st, stop=sp,
                            tile_position=(0, 0))
                        # block1: 64-wide diag at rows 64..127 -> psum 0..63
                        nc.tensor.matmul(
                            pv1[:, ksl], dwd1_sb[64:128, ti, :],
                            u1v[64:128, rsl, csl],
                            start=st, stop=sp,
                            tile_position=(64, 0))
                sl = slice(c * CHUNK, (c + 1) * CHUNK)
                nc.scalar.activation(out=v0[:, sl], in_=pv0, func=AF.Copy,
                                     accum_out=sumv0[:, c:c + 1])
                nc.scalar.activation(out=v1[:, sl], in_=pv1, func=AF.Copy,
                                     accum_out=sumv1[0:64, c:c + 1])

            # ---- phase D: BN2 -> c2; w = relu(v + c2) in-place;
            #      proj weights scaled by g2*rstd_v ----
            sq_v0 = chunk_sumsq("sqv0", v0, nc.vector)
            sq_v1 = chunk_sumsq("sqv1", v1, nc.vector)
            c2_0, rstdv0, _ = bn_prep("bn2b0", sumv0, sq_v0, eps_sb[0:128],
                                      r2b0)
            c2_1, rstdv1, _ = bn_prep("bn2b1", sumv1[0:64], sq_v1,
                                      eps_sb[0:64], r2b1)

            alpha0 = stats.tile([128, 1], f32, tag="alpha0", name="alpha0")
            nc.vector.tensor_mul(alpha0, g2b0, rstdv0)
            alpha1 = stats.tile([64, 1], f32, tag="alpha1", name="alpha1")
            nc.vector.tensor_mul(alpha1, g2b1, rstdv1)
            projs0 = stats.tile([128, COUT], bf, tag="projs0", name="projs0")
            nc.scalar.activation(out=projs0, in_=wproj0_sb, func=AF.Copy,
                                 scale=alpha0)
            projs1 = stats.tile([64, COUT], bf, tag="projs1", name="projs1")
            nc.scalar.activation(out=projs1, in_=wproj1_sb, func=AF.Copy,
                                 scale=alpha1)

            for q in range(4):  # 4096-slabs on DVE (4x perf mode)
                qs = slice(4096 * q, 4096 * (q + 1))
                nc.vector.tensor_scalar(out=v0[:, qs], in0=v0[:, qs],
                                        scalar1=c2_0, scalar2=0.0,
                                        op0=OP.add, op1=OP.max)
                nc.vector.tensor_scalar(out=v1[:, qs], in0=v1[:, qs],
                                        scalar1=c2_1, scalar2=0.0,
                                        op0=OP.add, op1=OP.max)

            # ---- phase E: proj conv (bf16) -> out3, evict + sums ----
            out3 = big.tile([64, HW], bf, tag="out3", name="out3")
            sumo = stats.tile([64, NCH], f32, tag="sumo", name="sumo")
            for c in range(NCH):
                po = psum.tile([64, CHUNK], f32, tag="ps", name="po")
                for k in range(CHUNK // 512):
                    sl = slice(c * CHUNK + k * 512, c * CHUNK + (k + 1) * 512)
                    nc.tensor.matmul(po[:, k * 512:(k + 1) * 512], projs0,
                                     v0[:, sl], start=True, stop=False,
                                     tile_position=(0, 0))
                    nc.tensor.matmul(po[:, k * 512:(k + 1) * 512], projs1,
                                     v1[:, sl], start=False, stop=True,
                                     tile_position=(0, 0))
                nc.scalar.activation(out=out3[:, c * CHUNK:(c + 1) * CHUNK],
                                     in_=po, func=AF.Copy,
                                     accum_out=sumo[:, c:c + 1])

            # ---- phase F: BN3 stats; final = relu(a3*out3 + b3 + x) ----
            sq_o = chunk_sumsq("sqo", out3, nc.vector)
            _, rstd3, mean3 = bn_prep("bn3", sumo, sq_o, eps_sb[0:64], g3)
            a3 = stats.tile([COUT, 1], f32, tag="a3", name="a3")
            nc.vector.tensor_mul(a3, g3, rstd3)
            t3 = stats.tile([COUT, 1], f32, tag="t3", name="t3")
            nc.vector.tensor_mul(t3, mean3, a3)
            b3f = stats.tile([COUT, 1], f32, tag="b3f", name="b3f")
            nc.vector.tensor_tensor(b3f, b3t, t3, op=OP.subtract)

            for c in range(NCH):
                xr = xin.tile([COUT, CHUNK], f32, tag="xp", name="xr")
                sl = slice(c * CHUNK, (c + 1) * CHUNK)
                nc.sync.dma_start(out=xr, in_=x_d[s, :, sl])
                nc.vector.affine_then_add(out=xr, in0=out3[:, sl], in1=xr,
                                          scale=a3, bias=b3f)
                nc.scalar.activation(out=xr, in_=xr, func=AF.Relu)
                nc.sync.dma_start(out=out_d[s, :, sl], in_=xr)

    nc.compile()
    return nc


def _get_program(reps=1):
    key = ("nc", reps)
    if key not in _PROG:
        _PROG[key] = _build_program(reps)
    return _PROG[key]


def _host_prep(x, device_ids, w_exp, g_exp, b_exp, w_dw, g_dw, b_dw,
               w_proj, g_proj, b_proj):
    """Build the per-core input maps (numpy only)."""
    f32 = np.float32
    ids = np.asarray(device_ids)
    ge = np.asarray(g_exp, f32)[:, :, 0, 0]   # [D, 192]
    be = np.asarray(b_exp, f32)[:, :, 0, 0]
    gd = np.asarray(g_dw, f32)[:, :, 0, 0]
    bd = np.asarray(b_dw, f32)[:, :, 0, 0]
    gp = np.asarray(g_proj, f32)[:, :, 0, 0]  # [D, 64]
    bp = np.asarray(b_proj, f32)[:, :, 0, 0]
    assert (ge > 0).all() and (gd > 0).all(), "relu-commute needs gamma>0"

    r1 = (be / ge)[ids]   # [B, 192]
    r2 = (bd / gd)[ids]
    g2 = gd[ids]
    g3 = gp[ids]          # [B, 64]
    b3 = bp[ids]

    wexp_lhsT = np.ascontiguousarray(
        np.asarray(w_exp, f32)[:, :, 0, 0].T).astype(BF16)  # [64, 192]
    wp = np.asarray(w_proj, f32)[:, :, 0, 0]  # [64, 192]
    wproj_lhsT0 = np.ascontiguousarray(wp[:, 0:128].T).astype(f32)  # [128,64]
    wproj_lhsT1 = np.ascontiguousarray(wp[:, 128:192].T).astype(f32)  # [64,64]

    dw = np.asarray(w_dw, f32)[:, 0, :, :]  # [192, 3, 3]
    dw_diag0 = np.zeros((128, 9, 128), f32)
    dw_diag1 = np.zeros((64, 9, 64), f32)
    i128 = np.arange(128)
    i64 = np.arange(64)
    for ti, (dy, dx) in enumerate(TAPS):
        dw_diag0[i128, ti, i128] = dw[i128, dy + 1, dx + 1]
        dw_diag1[i64, ti, i64] = dw[128 + i64, dy + 1, dx + 1]
    dw_diag0 = dw_diag0.astype(BF16)
    dw_diag1 = dw_diag1.astype(BF16)

    xf = np.asarray(x, f32).reshape(B, CIN, HW)

    in_maps = []
    for core in range(NCORES):
        sl = slice(core * BLOC, (core + 1) * BLOC)
        in_maps.append({
            "x": np.ascontiguousarray(xf[sl]),
            "wexp_lhsT": wexp_lhsT,
            "dw_diagF0": dw_diag0,
            "dw_diagF1": dw_diag1,
            "wproj_lhsT0": wproj_lhsT0,
            "wproj_lhsT1": wproj_lhsT1,
            "r1b0": np.ascontiguousarray(r1[sl, 0:128], f32),
            "r1b1": np.ascontiguousarray(r1[sl, 128:192], f32),
            "r2b0": np.ascontiguousarray(r2[sl, 0:128], f32),
            "r2b1": np.ascontiguousarray(r2[sl, 128:192], f32),
            "g2b0": np.ascontiguousarray(g2[sl, 0:128], f32),
            "g2b1": np.ascontiguousarray(g2[sl, 128:192], f32),
            "g3": np.ascontiguousarray(g3[sl], f32),
            "b3": np.ascontiguousarray(b3[sl], f32),
        })
    return in_maps


def kernel(x, device_ids, w_exp, g_exp, b_exp, w_dw, g_dw, b_dw,
           w_proj, g_proj, b_proj, _trace=False, _tmpdir=None):
    from concourse import bass_utils

    nc = _get_program()
    in_maps = _host_prep(x, device_ids, w_exp, g_exp, b_exp, w_dw, g_dw,
                         b_dw, w_proj, g_proj, b_proj)
    res = bass_utils.run_bass_kernel_spmd(
        nc, in_maps, core_ids=list(range(NCORES)), trace=_trace,
        tmpdir=_tmpdir)
    out = np.stack([r["out"] for r in res.results], axis=0)
    out = out.reshape(B, COUT, H, W).astype(np.float32)
    if _trace:
        kernel._last_results = res
    return out



# revision 13
# speedup vs baseline: 2.5368x; 1.0870x over previous
"""Trainium2 Bass kernel for nn_Block_57861799412251.

CondBN inverted-residual block:
  1x1 conv (64->192) -> per-sample BN + ReLU
  depthwise 3x3      -> per-sample BN + ReLU
  1x1 conv (192->64) -> per-sample BN
  + identity shortcut -> ReLU

Sharding: data-parallel over batch (32 samples / 8 cores = 4 per core).

Key algebra (per-sample, per-channel BN with gamma>0):
  relu(g*(z-mu)/sd + b) = (g/sd) * relu(z + (sd*b/g - mu))
so each BN+ReLU collapses to a per-channel bias add + relu, with the
positive per-channel scale (g/sd) either cancelled by the next BN's
normalization (BN1, BN2) or folded into the next matmul's weights (BN2
-> proj weights).  BN3's affine is applied in the final residual op.

Layout: channels on partitions, spatial (128*128=16384) on the free axis.
conv1/proj matmuls in float32r, depthwise diag-matmuls in bf16.

The depthwise 3x3 runs as 9 PSUM-accumulated 32x32 diag-matmuls per
32-channel group over a zero-padded [C, 130, 130] bf16 layout, with six
disjoint (row,col) PE tile regions streaming concurrently:
  block0 (ch 0..127):   groups at tiles (0,0) (32,32) (64,64) (96,96)
  block1 (ch 128..191): z1/u1 live on partitions 64..127 (placed there by
  conv1's output-column position); dw tiles (64,0) (96,32) map them back
  to partitions 0..63 where v1/w1/proj live.
"""

import sys

sys.path.insert(0, "/opt/trn_rl_repo")

import numpy as np
import ml_dtypes

BF16 = ml_dtypes.bfloat16

B, CIN, H, W = 32, 64, 128, 128
HW = H * W
CEXP, COUT, D = 192, 64, 6
NCORES = 8
BLOC = B // NCORES  # 4 samples per core
HP, WP = H + 2, W + 2  # padded spatial for depthwise conv
PADHW = HP * WP
EPS = 1e-5
VAR_CORR = HW / (HW - 1.0)  # torch-style unbiased variance
CHUNK = 1024  # PSUM chunk (2 banks)
NCH = HW // CHUNK  # 16
SQP = 2048  # sumsq STT piece
TAPS = [(dy, dx) for dy in (-1, 0, 1) for dx in (-1, 0, 1)]
TAPS_PE = [(dy, dx) for dy in (-1, 1) for dx in (-1, 0, 1)]  # 6 on PE
# dy=0 row runs as 3 in-place STT taps on the vector engine

_PROG = {}


def _build_program(reps=1):
    import concourse.bass as bass
    import concourse.bacc as bacc
    import concourse.tile as tile
    import concourse.mybir as mybir
    from contextlib import ExitStack

    dt = mybir.dt
    AF = mybir.ActivationFunctionType
    OP = mybir.AluOpType

    nc = bacc.Bacc("TRN2", target_bir_lowering=False, debug=False,
                   num_devices=NCORES)

    f32 = dt.float32
    f32r = dt.float32r
    bf = dt.bfloat16

    x_d = nc.dram_tensor("x", [BLOC, CIN, HW], f32, kind="ExternalInput").ap()
    out_d = nc.dram_tensor("out", [BLOC, COUT, HW], f32,
                           kind="ExternalOutput").ap()
    wexp_d = nc.dram_tensor("wexp_lhsT", [CIN, CEXP], bf,
                            kind="ExternalInput").ap()
    # full-width diag matrices per tap: [c, t, c] = w_dw[c, tap t]
    dwd0_d = nc.dram_tensor("dw_diagF0", [128, 9, 128], bf,
                            kind="ExternalInput").ap()
    # block1 diag64 per tap: [c, t, c] = w_dw[128+c, tap t]
    dwd1_d = nc.dram_tensor("dw_diagF1", [64, 9, 64], bf,
                            kind="ExternalInput").ap()
    dvw0_d = nc.dram_tensor("dv_w0", [128, 3], f32,
                            kind="ExternalInput").ap()
    dvw1_d = nc.dram_tensor("dv_w1", [64, 3], f32,
                            kind="ExternalInput").ap()
    wproj0_d = nc.dram_tensor("wproj_lhsT0", [128, COUT], f32,
                              kind="ExternalInput").ap()
    wproj1_d = nc.dram_tensor("wproj_lhsT1", [64, COUT], f32,
                              kind="ExternalInput").ap()
    # per-sample per-channel tables; block0 = ch 0:128, block1 = ch 128:192
    r1b0_d = nc.dram_tensor("r1b0", [BLOC, 128], f32, kind="ExternalInput").ap()
    r1b1_d = nc.dram_tensor("r1b1", [BLOC, 64], f32, kind="ExternalInput").ap()
    r2b0_d = nc.dram_tensor("r2b0", [BLOC, 128], f32, kind="ExternalInput").ap()
    r2b1_d = nc.dram_tensor("r2b1", [BLOC, 64], f32, kind="ExternalInput").ap()
    g2b0_d = nc.dram_tensor("g2b0", [BLOC, 128], f32, kind="ExternalInput").ap()
    g2b1_d = nc.dram_tensor("g2b1", [BLOC, 64], f32, kind="ExternalInput").ap()
    g3_d = nc.dram_tensor("g3", [BLOC, COUT], f32, kind="ExternalInput").ap()
    b3_d = nc.dram_tensor("b3", [BLOC, COUT], f32, kind="ExternalInput").ap()

    with ExitStack() as ctx:
        tc = ctx.enter_context(tile.TileContext(nc))
        const = ctx.enter_context(tc.tile_pool(name="const", bufs=1))
        stats = ctx.enter_context(tc.tile_pool(name="stats", bufs=2))
        big = ctx.enter_context(tc.tile_pool(name="big", bufs=1))
        xin = ctx.enter_context(tc.tile_pool(name="xin", bufs=4))
        psum = ctx.enter_context(tc.tile_pool(name="psum", bufs=4,
                                              space="PSUM"))

        # ---- constants ----
        wexp_sb = const.tile([CIN, CEXP], bf)
        nc.sync.dma_start(out=wexp_sb, in_=wexp_d)
        dwd0_sb = const.tile([128, 9, 128], bf)
        nc.sync.dma_start(out=dwd0_sb, in_=dwd0_d)
        # block1 diags live at partitions 64..127 (tile row base 64)
        dwd1_sb = const.tile([128, 9, 64], bf)
        nc.sync.dma_start(out=dwd1_sb[64:128], in_=dwd1_d)
        wproj0_sb = const.tile([128, COUT], f32)
        nc.sync.dma_start(out=wproj0_sb, in_=wproj0_d)
        wproj1_sb = const.tile([128, COUT], f32)
        nc.sync.dma_start(out=wproj1_sb[64:128], in_=wproj1_d)
        dvw0_sb = const.tile([128, 3], f32)
        nc.sync.dma_start(out=dvw0_sb, in_=dvw0_d)
        dvw1_sb = const.tile([128, 3], f32)
        nc.sync.dma_start(out=dvw1_sb[64:128], in_=dvw1_d)
        eps_sb = const.tile([128, 1], f32)
        nc.vector.memset(eps_sb, EPS)

        # padded u buffers (borders stay zero forever).
        # u0: ch 0..127 on partitions 0..127; u1: ch 128..191 on 64..127.
        u0_sb = const.tile([128, PADHW], bf)
        nc.gpsimd.memset(u0_sb, 0.0)
        u1_sb = const.tile([128, PADHW], bf)
        nc.gpsimd.memset(u1_sb, 0.0)
        u0v = u0_sb.rearrange("p (h w) -> p h w", h=HP)
        u1v = u1_sb.rearrange("p (h w) -> p h w", h=HP)

        loop_ctx = tc.For_i(0, reps, 1) if reps > 1 else None
        if loop_ctx is not None:
            ctx.enter_context(loop_ctx)

        def chunk_sumsq(name, src, engine, piece=SQP):
            """accumulate sum(src^2) over the free axis via chunked STT.
            src: [P, HW] bf16 AP (any base partition); all operand tiles
            are placed at src's base partition."""
            P = src.partition_size()
            lo = src.base_partition()
            n = HW // piece
            acc = stats.tile([128, n], f32, tag=f"{name}_acc",
                             name=f"{name}_acc")[lo:lo + P]
            for i in range(n):
                scr = big.tile([128, piece], bf, tag="scr", bufs=1,
                               name=f"{name}_scr")
                engine.scalar_tensor_tensor(
                    out=scr[lo:lo + P],
                    in0=src[:, i * piece:(i + 1) * piece],
                    scalar=1.0,
                    in1=src[:, i * piece:(i + 1) * piece],
                    op0=OP.bypass,
                    op1=OP.mult,
                    accum_out=acc[:, i:i + 1],
                )
            tot = stats.tile([128, 1], f32, tag=f"{name}_tot",
                             name=f"{name}_tot")[lo:lo + P]
            nc.vector.tensor_reduce(tot, acc, axis=mybir.AxisListType.X,
                                    op=OP.add)
            return tot

        def bn_prep(name, sum_parts, sumsq, eps_ap, r_ap):
            """Produce (c = sd*r - mean, rstd, mean) for a [P,1] stat lane
            set. sum_parts: [P, n] per-chunk sums; sumsq: [P,1].  All tiles
            are placed at sum_parts' base partition."""
            P = sum_parts.partition_size()
            lo = sum_parts.base_partition()

            def stile(suffix):
                return stats.tile([128, 1], f32, tag=f"{name}_{suffix}",
                                  name=f"{name}_{suffix}")[lo:lo + P]

            s = stile("s")
            nc.vector.tensor_reduce(s, sum_parts, axis=mybir.AxisListType.X,
                                    op=OP.add)
            mean = stile("mean")
            nc.vector.tensor_scalar(out=mean, in0=s, scalar1=1.0 / HW,
                                    scalar2=None, op0=OP.mult)
            ex2 = stile("ex2")
            nc.vector.tensor_scalar(out=ex2, in0=sumsq, scalar1=1.0 / HW,
                                    scalar2=None, op0=OP.mult)
            var = stile("var")
            nc.vector.scalar_tensor_tensor(out=var, in0=mean, scalar=mean,
                                           in1=ex2, op0=OP.mult,
                                           op1=OP.subtract)
            nc.vector.tensor_scalar(out=var, in0=var, scalar1=-1.0,
                                    scalar2=None, op0=OP.mult)
            sd = stile("sd")
            nc.scalar.activation(out=sd, in_=var, func=AF.Sqrt,
                                 bias=eps_ap, scale=VAR_CORR)
            rstd = stile("rstd")
            nc.vector.reciprocal(rstd, sd)
            c = stile("c")
            nc.vector.scalar_tensor_tensor(out=c, in0=sd, scalar=r_ap,
                                           in1=mean, op0=OP.mult,
                                           op1=OP.subtract)
            return c, rstd, mean

        for s in range(BLOC):
            # ---- per-sample params.  Block1 z-side tables live on
            # partitions 64..127; v-side (r2/g2) on 0..63. ----
            def ld(name, dram_ap, lo, P):
                t = stats.tile([128, 1], f32, tag=f"p_{name}",
                               name=f"p_{name}")
                nc.sync.dma_start(out=t[lo:lo + P], in_=dram_ap[s, :, None])
                return t[lo:lo + P]

            r1b0 = ld("r1b0", r1b0_d, 0, 128)
            r1b1 = ld("r1b1", r1b1_d, 64, 64)   # z-side: partitions 64..127
            r2b0 = ld("r2b0", r2b0_d, 0, 128)
            r2b1 = ld("r2b1", r2b1_d, 64, 64)   # v-side: partitions 64..127
            g2b0 = ld("g2b0", g2b0_d, 0, 128)
            g2b1 = ld("g2b1", g2b1_d, 64, 64)
            g3 = ld("g3", g3_d, 0, COUT)
            b3t = ld("b3", b3_d, 0, COUT)

            # ---- phase A: conv1 (f32r, straight from streamed x) ----
            z0 = big.tile([128, HW], bf, tag="zv0", name="z0")
            z1t = big.tile([128, HW], bf, tag="zv1", name="z1t")
            z1 = z1t[64:128]  # ch 128..191 on partitions 64..127
            sumz0 = stats.tile([128, NCH], f32, tag="sumz0", name="sumz0")
            sumz1 = stats.tile([128, NCH], f32, tag="sumz1", name="sumz1")
            for c in range(NCH):  # 16 chunks of 1024
                xp = xin.tile([CIN, CHUNK], f32, tag="xp", name="xp")
                nc.sync.dma_start(out=xp,
                                  in_=x_d[s, :, c * CHUNK:(c + 1) * CHUNK])
                xbf = xin.tile([CIN, CHUNK], bf, tag="xbf", name="xbf")
                nc.scalar.activation(out=xbf, in_=xp, func=AF.Copy)
                pz0 = psum.tile([128, CHUNK], f32, tag="ps", name="pz0")
                pz1 = psum.tile([128, CHUNK], f32, tag="ps", name="pz1")
                for k in range(CHUNK // 512):
                    rhs = xbf[:, k * 512:(k + 1) * 512]
                    nc.tensor.matmul(pz0[:, k * 512:(k + 1) * 512],
                                     wexp_sb[:, 0:128], rhs,
                                     start=True, stop=True,
                                     tile_position=(0, 0))
                    nc.tensor.matmul(pz1[64:128, k * 512:(k + 1) * 512],
                                     wexp_sb[:, 128:CEXP], rhs,
                                     start=True, stop=True,
                                     tile_position=(0, 64))
                sl = slice(c * CHUNK, (c + 1) * CHUNK)
                nc.scalar.activation(out=z0[:, sl], in_=pz0, func=AF.Copy,
                                     accum_out=sumz0[:, c:c + 1])
                nc.scalar.activation(out=z1[:, sl], in_=pz1[64:128],
                                     func=AF.Copy,
                                     accum_out=sumz1[64:128, c:c + 1])

            # ---- phase B: BN1 stats -> c1; u = relu(z + c1) ----
            sq_z0 = chunk_sumsq("sqz0", z0, nc.vector)
            sq_z1 = chunk_sumsq("sqz1", z1, nc.vector)
            c1_0, _, _ = bn_prep("bn1b0", sumz0, sq_z0, eps_sb[0:128], r1b0)
            c1_1, _, _ = bn_prep("bn1b1", sumz1[64:128], sq_z1,
                                 eps_sb[64:128], r1b1)

            z0v = z0.rearrange("p (h w) -> p h w", h=H)
            z1v = z1.rearrange("p (h w) -> p h w", h=H)
            for q in range(4):  # 32-row slabs on DVE (4x perf mode)
                rs = slice(1 + 32 * q, 1 + 32 * (q + 1))
                zs = slice(32 * q, 32 * (q + 1))
                nc.vector.tensor_scalar(out=u0v[:, rs, 1:W + 1],
                                        in0=z0v[:, zs, :],
                                        scalar1=c1_0, scalar2=0.0,
                                        op0=OP.add, op1=OP.max)
                nc.vector.tensor_scalar(out=u1v[64:128, rs, 1:W + 1],
                                        in0=z1v[:, zs, :],
                                        scalar1=c1_1, scalar2=0.0,
                                        op0=OP.add, op1=OP.max)

            # ---- phase C: depthwise 3x3 -> v, 6 concurrent PE tile slots ----
            v0 = big.tile([128, HW], bf, tag="zv0", name="v0")
            v1t = big.tile([128, HW], bf, tag="zv1", name="v1t")
            v1 = v1t[64:128]  # ch 128..191 stay on partitions 64..127
            sumv0p = stats.tile([128, 4], f32, tag="sumv0", name="sumv0p")
            sumv1p = stats.tile([128, 4], f32, tag="sumv1", name="sumv1p")
            rows_per_512 = 512 // W  # 4
            for c in range(NCH):
                pv0 = psum.tile([128, CHUNK], f32, tag="ps", name="pv0")
                pv1t = psum.tile([128, CHUNK], f32, tag="ps", name="pv1")
                pv1 = pv1t[64:128]
                for k in range(CHUNK // 512):
                    h0 = (c * CHUNK + k * 512) // W
                    ksl = slice(k * 512, (k + 1) * 512)
                    for ti, (dy, dx) in enumerate(TAPS_PE):
                        rsl = slice(1 + h0 + dy, 1 + h0 + dy + rows_per_512)
                        csl = slice(1 + dx, 1 + dx + W)
                        st, sp = (ti == 0), (ti == 5)
                        t9 = TAPS.index((dy, dx))
                        # block0: one full 128-wide diag matmul at (0,0)
                        nc.tensor.matmul(
                            pv0[:, ksl], dwd0_sb[:, t9, :],
                            u0v[:, rsl, csl],
                            start=st, stop=sp,
                            tile_position=(0, 0))
                        # block1: 64-diag at rows 64..127 -> psum 64..127
                        nc.tensor.matmul(
                            pv1[:, ksl], dwd1_sb[64:128, t9, :],
                            u1v[64:128, rsl, csl],
                            start=st, stop=sp,
                            tile_position=(64, 64))
                sl = slice(c * CHUNK, (c + 1) * CHUNK)
                nc.scalar.activation(out=v0[:, sl], in_=pv0, func=AF.Copy)
                nc.scalar.activation(out=v1[:, sl], in_=pv1, func=AF.Copy)

            # dy=0 taps on the vector engine, in-place over v; the last
            # tap's accumulator yields sum(v) for BN2.
            v0v3 = v0.rearrange("p (h w) -> p h w", h=H)
            v1v3 = v1t.rearrange("p (h w) -> p h w", h=H)
            for t in range(3):
                dx = t - 1
                last = (t == 2)
                csl = slice(1 + dx, 1 + dx + W)
                for q in range(4):
                    rs = slice(32 * q, 32 * (q + 1))
                    urs = slice(1 + 32 * q, 1 + 32 * (q + 1))
                    nc.vector.scalar_tensor_tensor(
                        out=v0v3[:, rs, :], in0=u0v[:, urs, csl],
                        scalar=dvw0_sb[:, t:t + 1], in1=v0v3[:, rs, :],
                        op0=OP.mult, op1=OP.add,
                        accum_out=(sumv0p[:, q:q + 1] if last else None))
                    nc.vector.scalar_tensor_tensor(
                        out=v1v3[64:128, rs, :], in0=u1v[64:128, urs, csl],
                        scalar=dvw1_sb[64:128, t:t + 1],
                        in1=v1v3[64:128, rs, :],
                        op0=OP.mult, op1=OP.add,
                        accum_out=(sumv1p[64:128, q:q + 1] if last else None))

            # ---- phase D: BN2 -> c2; w = relu(v + c2) in-place;
            #      proj weights scaled by g2*rstd_v ----
            sq_v0 = chunk_sumsq("sqv0", v0, nc.vector)
            sq_v1 = chunk_sumsq("sqv1", v1, nc.vector)
            c2_0, rstdv0, _ = bn_prep("bn2b0", sumv0p, sq_v0, eps_sb[0:128],
                                      r2b0)
            c2_1, rstdv1, _ = bn_prep("bn2b1", sumv1p[64:128], sq_v1,
                                      eps_sb[64:128], r2b1)

            alpha0 = stats.tile([128, 1], f32, tag="alpha0", name="alpha0")
            nc.vector.tensor_mul(alpha0, g2b0, rstdv0)
            alpha1 = stats.tile([128, 1], f32, tag="alpha1",
                                name="alpha1")[64:128]
            nc.vector.tensor_mul(alpha1, g2b1, rstdv1)
            projs0 = stats.tile([128, COUT], bf, tag="projs0", name="projs0")
            nc.scalar.activation(out=projs0, in_=wproj0_sb, func=AF.Copy,
                                 scale=alpha0)
            projs1 = stats.tile([128, COUT], bf, tag="projs1",
                                name="projs1")[64:128]
            nc.scalar.activation(out=projs1, in_=wproj1_sb[64:128],
                                 func=AF.Copy, scale=alpha1)

            for q in range(4):  # 4096-slabs on DVE (4x perf mode)
                qs = slice(4096 * q, 4096 * (q + 1))
                nc.vector.tensor_scalar(out=v0[:, qs], in0=v0[:, qs],
                                        scalar1=c2_0, scalar2=0.0,
                                        op0=OP.add, op1=OP.max)
                nc.vector.tensor_scalar(out=v1[:, qs], in0=v1[:, qs],
                                        scalar1=c2_1, scalar2=0.0,
                                        op0=OP.add, op1=OP.max)

            # ---- phase E: proj conv (bf16) -> out3, evict + sums ----
            out3 = big.tile([64, HW], bf, tag="out3", name="out3")
            sumo = stats.tile([64, NCH], f32, tag="sumo", name="sumo")
            for c in range(NCH):
                po = psum.tile([64, CHUNK], f32, tag="ps", name="po")
                for k in range(CHUNK // 512):
                    sl = slice(c * CHUNK + k * 512, c * CHUNK + (k + 1) * 512)
                    nc.tensor.matmul(po[:, k * 512:(k + 1) * 512], projs0,
                                     v0[:, sl], start=True, stop=False,
                                     tile_position=(0, 0))
                    nc.tensor.matmul(po[:, k * 512:(k + 1) * 512], projs1,
                                     v1[:, sl], start=False, stop=True,
                                     tile_position=(64, 0))
                nc.scalar.activation(out=out3[:, c * CHUNK:(c + 1) * CHUNK],
                                     in_=po, func=AF.Copy,
                                     accum_out=sumo[:, c:c + 1])

            # ---- phase F: BN3 stats; final = relu(a3*out3 + b3 + x) ----
            sq_o = chunk_sumsq("sqo", out3, nc.vector)
            _, rstd3, mean3 = bn_prep("bn3", sumo, sq_o, eps_sb[0:64], g3)
            a3 = stats.tile([COUT, 1], f32, tag="a3", name="a3")
            nc.vector.tensor_mul(a3, g3, rstd3)
            t3 = stats.tile([COUT, 1], f32, tag="t3", name="t3")
            nc.vector.tensor_mul(t3, mean3, a3)
            b3f = stats.tile([COUT, 1], f32, tag="b3f", name="b3f")
            nc.vector.tensor_tensor(b3f, b3t, t3, op=OP.subtract)

            for c in range(NCH):
                xr = xin.tile([COUT, CHUNK], f32, tag="xp", name="xr")
                sl = slice(c * CHUNK, (c + 1) * CHUNK)
                nc.sync.dma_start(out=xr, in_=x_d[s, :, sl])
                nc.vector.affine_then_add(out=xr, in0=out3[:, sl], in1=xr,
                                          scale=a3, bias=b3f)
                nc.scalar.activation(out=xr, in_=xr, func=AF.Relu)
                nc.sync.dma_start(out=out_d[s, :, sl], in_=xr)

    nc.compile()
    return nc


def _get_program(reps=1):
    key = ("nc", reps)
    if key not in _PROG:
        _PROG[key] = _build_program(reps)
    return _PROG[key]


def _host_prep(x, device_ids, w_exp, g_exp, b_exp, w_dw, g_dw, b_dw,
               w_proj, g_proj, b_proj):
    """Build the per-core input maps (numpy only)."""
    f32 = np.float32
    ids = np.asarray(device_ids)
    ge = np.asarray(g_exp, f32)[:, :, 0, 0]   # [D, 192]
    be = np.asarray(b_exp, f32)[:, :, 0, 0]
    gd = np.asarray(g_dw, f32)[:, :, 0, 0]
    bd = np.asarray(b_dw, f32)[:, :, 0, 0]
    gp = np.asarray(g_proj, f32)[:, :, 0, 0]  # [D, 64]
    bp = np.asarray(b_proj, f32)[:, :, 0, 0]
    assert (ge > 0).all() and (gd > 0).all(), "relu-commute needs gamma>0"

    r1 = (be / ge)[ids]   # [B, 192]
    r2 = (bd / gd)[ids]
    g2 = gd[ids]
    g3 = gp[ids]          # [B, 64]
    b3 = bp[ids]

    wexp_lhsT = np.ascontiguousarray(
        np.asarray(w_exp, f32)[:, :, 0, 0].T).astype(BF16)  # [64, 192]
    wp = np.asarray(w_proj, f32)[:, :, 0, 0]  # [64, 192]
    wproj_lhsT0 = np.ascontiguousarray(wp[:, 0:128].T).astype(f32)  # [128,64]
    wproj_lhsT1 = np.ascontiguousarray(wp[:, 128:192].T).astype(f32)  # [64,64]

    dw = np.asarray(w_dw, f32)[:, 0, :, :]  # [192, 3, 3]
    dw_diag0 = np.zeros((128, 9, 128), f32)
    dw_diag1 = np.zeros((64, 9, 64), f32)
    i128 = np.arange(128)
    i64 = np.arange(64)
    for ti, (dy, dx) in enumerate(TAPS):
        dw_diag0[i128, ti, i128] = dw[i128, dy + 1, dx + 1]
        dw_diag1[i64, ti, i64] = dw[128 + i64, dy + 1, dx + 1]
    dw_diag0 = dw_diag0.astype(BF16)
    dw_diag1 = dw_diag1.astype(BF16)
    dv_w0 = np.ascontiguousarray(dw[0:128, 1, :], f32)   # [128, 3]
    dv_w1 = np.ascontiguousarray(dw[128:192, 1, :], f32)  # [64, 3]

    xf = np.asarray(x, f32).reshape(B, CIN, HW)

    in_maps = []
    for core in range(NCORES):
        sl = slice(core * BLOC, (core + 1) * BLOC)
        in_maps.append({
            "x": np.ascontiguousarray(xf[sl]),
            "wexp_lhsT": wexp_lhsT,
            "dw_diagF0": dw_diag0,
            "dw_diagF1": dw_diag1,
            "dv_w0": dv_w0,
            "dv_w1": dv_w1,
            "wproj_lhsT0": wproj_lhsT0,
            "wproj_lhsT1": wproj_lhsT1,
            "r1b0": np.ascontiguousarray(r1[sl, 0:128], f32),
            "r1b1": np.ascontiguousarray(r1[sl, 128:192], f32),
            "r2b0": np.ascontiguousarray(r2[sl, 0:128], f32),
            "r2b1": np.ascontiguousarray(r2[sl, 128:192], f32),
            "g2b0": np.ascontiguousarray(g2[sl, 0:128], f32),
            "g2b1": np.ascontiguousarray(g2[sl, 128:192], f32),
            "g3": np.ascontiguousarray(g3[sl], f32),
            "b3": np.ascontiguousarray(b3[sl], f32),
        })
    return in_maps


def kernel(x, device_ids, w_exp, g_exp, b_exp, w_dw, g_dw, b_dw,
           w_proj, g_proj, b_proj, _trace=False, _tmpdir=None):
    from concourse import bass_utils

    nc = _get_program()
    in_maps = _host_prep(x, device_ids, w_exp, g_exp, b_exp, w_dw, g_dw,
                         b_dw, w_proj, g_proj, b_proj)
    res = bass_utils.run_bass_kernel_spmd(
        nc, in_maps, core_ids=list(range(NCORES)), trace=_trace,
        tmpdir=_tmpdir)
    out = np.stack([r["out"] for r in res.results], axis=0)
    out = out.reshape(B, COUT, H, W).astype(np.float32)
    if _trace:
        kernel._last_results = res
    return out

